# revision 43
# baseline (speedup 1.0000x reference)
"""Chamfer-distance-with-normals Trainium2 kernel.

Sharding: data-parallel over batch B=8 across the 8 NeuronCores (one batch
element per core). Per core, the 4096x4096 squared-distance matrix
D[n, m] = |x1|^2 + |x2|^2 - 2*x1.x2 is produced one 128-row block at a time
and never materialized.

Matmuls use a bf16x3 decomposition (each fp32 operand split into three bf16
terms; the 6 dominant cross products + rank-1 norm terms give K=24 exact
bf16*bf16 products accumulated in fp32 PSUM, |D err| ~ 7e-6) so the PE runs
at 1 cycle/row instead of fp32's 4, packed 4x into 32-row PE groups.

Row argmin via two single-pass streaming ops per 128-row block:
  1. VectorE tensor_tensor_scan runs a TWO-STREAM prefix min (data0 = PSUM
     half m in [0,2048), data1 = ScalarE-copied SBUF half [2048,4096)):
     state = min(min(state, lo[t]), hi[t]).  2048 steps cover 4096 columns.
  2. ScalarE activation(Sign, scale=-1, bias=rowmin, accum_out) counts
     prefix entries strictly above the min: accum = -t* where t* is the
     first step at which the running min reaches the row min.  The argmin is
     then one of {t*, t*+2048}; the host resolves the pair (and computes
     exact distances and the O(B*N) normal losses) in float64.
"""

import functools
from contextlib import ExitStack

import ml_dtypes
import numpy as np

import concourse.bass as bass
import concourse.mybir as mybir
import concourse.tile as tile
from concourse import bacc
from concourse.bass_utils import run_bass_kernel_spmd

B = 8
N_PTS = 4096
P = 128
K_ROWS = 24  # bf16x3 decomposition rows


def build_chamfer(n_pts=N_PTS, m_tile=512):
    """Build the Bass program. Returns the compiled Bacc module."""
    nc = bacc.Bacc("TRN2", target_bir_lowering=False, debug=False, num_devices=B)
    dt = mybir.dt

    n_blocks = n_pts // P
    half = n_pts // 2
    m_tile = min(m_tile, half)
    m_tiles = half // m_tile  # matmuls per half
    n_grp = min(4, m_tiles)  # concurrent PE row-groups

    ins = {}
    for name in ("s1", "t2", "s2", "t1"):
        ins[name] = nc.dram_tensor(
            name, [32 * min(4, half // m_tile), n_pts], dt.bfloat16, kind="ExternalInput"
        ).ap()
    outs = {}
    for name in ("c1", "c2"):
        outs[name] = nc.dram_tensor(name, [P, n_blocks], dt.float32, kind="ExternalOutput").ap()

    with tile.TileContext(nc) as tc, ExitStack() as ctx:
        const_pool = ctx.enter_context(tc.tile_pool(name="const", bufs=1))
        s_pool = ctx.enter_context(tc.tile_pool(name="srow", bufs=5))
        h1_pool = ctx.enter_context(tc.tile_pool(name="h1", bufs=4))
        psa_pool = ctx.enter_context(tc.tile_pool(name="psa", bufs=1, space="PSUM"))
        psb_pool = ctx.enter_context(tc.tile_pool(name="psb", bufs=1, space="PSUM"))
        res_pool = ctx.enter_context(tc.tile_pool(name="res", bufs=1))

        # Operands arrive from the host already replicated at base partitions
        # 0/32/64/96 so up to 4 matmuls run concurrently in distinct PE
        # row-groups (K=24 <= 32) after a single DMA.  Pass 2's loads are
        # deferred (emitted inside its pass) so they don't delay pass 1.
        op_tiles = {}

        def load_operand(name):
            t = const_pool.tile([32 * n_grp, n_pts], dt.bfloat16, tag=name)
            nc.sync.dma_start(t[:], ins[name][:])
            op_tiles[name] = t

        load_operand("s1")
        load_operand("t2")
        junk = const_pool.tile([P, half // 4], dt.float32, tag="junk")

        def emit_count(s_row, cnt_ap):
            # accum = sum(sign(rowmin - S[::4])) = -ceil(t*/4) (sign(0)==0);
            # the bias reads the row min straight from the scan's last column.
            # Counting the stride-4 subsample quarters ScalarE time; the host
            # resolves the remaining {4c-3..4c} x {lo, hi} candidates.
            s_even = s_row[:].rearrange("p (k four) -> p k four", four=4)[:, :, 0]
            nc.scalar.activation(
                junk[:],
                s_even,
                mybir.ActivationFunctionType.Sign,
                bias=s_row[:, half - 1 : half],
                scale=-1.0,
                accum_out=cnt_ap,
            )

        for pno, (lhs_name, rhs_name, cname) in enumerate(
            (("s1", "t2", "c1"), ("s2", "t1", "c2"))
        ):
            if lhs_name not in op_tiles:
                load_operand(lhs_name)
            if rhs_name not in op_tiles:
                load_operand(rhs_name)
            lhs = op_tiles[lhs_name]
            rhs = op_tiles[rhs_name]
            cnts = res_pool.tile([P, n_blocks], dt.float32, tag=f"c{pno}")

            def mm_half(ps, i, h):
                insts = []
                for jj in range(m_tiles):
                    j = h * m_tiles + jj
                    g = jj % n_grp
                    insts.append(nc.tensor.matmul(
                        ps[:, jj * m_tile : (jj + 1) * m_tile],
                        lhs[32 * g : 32 * g + K_ROWS, i * P : (i + 1) * P],
                        rhs[32 * g : 32 * g + K_ROWS, j * m_tile : (j + 1) * m_tile],
                        start=True,
                        stop=True,
                        tile_position=(32 * g, 0),
                    ))
                return insts

            pending = []  # (s_row, count slice) queue, lagging two blocks
            for i in range(n_blocks):
                # Blocks alternate between two 4-bank PSUM pools; within a
                # block both halves share one pool sequentially (hi matmuls ->
                # ACT copy drains -> lo matmuls reuse the banks).  The scan's
                # banks for block i+1 are therefore freed by block i's COPY,
                # not its scan, so consecutive scans run back-to-back.
                pool = psa_pool if i % 2 == 0 else psb_pool
                ps_hi = pool.tile([P, half], dt.float32, tag="ps")
                mm_half(ps_hi, i, 1)
                h1_sb = h1_pool.tile([P, half], dt.float32, tag="h1")
                nc.scalar.copy(h1_sb[:], ps_hi[:])
                ps_lo = pool.tile([P, half], dt.float32, tag="ps")
                mm_half(ps_lo, i, 0)
                # two-stream prefix min over (lo[t], hi[t]=lo[t]+half) pairs
                s_row = s_pool.tile([P, half], dt.float32, tag="s")
                nc.vector.tensor_tensor_scan(
                    s_row[:],
                    ps_lo[:],
                    h1_sb[:],
                    3.0e38,
                    op0=mybir.AluOpType.min,
                    op1=mybir.AluOpType.min,
                )
                # lag counts two blocks so no queued ScalarE op ever blocks
                # the copy feeding the next scan (ACT is strict FIFO)
                pending.append((s_row, cnts[:, i : i + 1]))
                if len(pending) > 2:
                    emit_count(*pending.pop(0))
            for p in pending:
                emit_count(*p)

            nc.sync.dma_start(outs[cname][:], cnts[:])

    nc.compile()
    return nc


@functools.lru_cache(maxsize=1)
def _compiled():
    return build_chamfer()


def _bf3(v):
    h = v.astype(ml_dtypes.bfloat16).astype(np.float32)
    r = v - h
    m = r.astype(ml_dtypes.bfloat16).astype(np.float32)
    l = (r - m).astype(ml_dtypes.bfloat16)
    return h.astype(ml_dtypes.bfloat16), m.astype(ml_dtypes.bfloat16), l


def _operands(xyz):
    """[n,3] fp32 -> (S, T) [24,n] bf16 stationary/moving operand rows.

    Row pairing (S row k multiplies T row k): per dim the 6 dominant bf16x3
    cross terms (hh, hm, mh, hl, lh, mm), then ones x (-sq h/m/l) and
    (-sq h/m/l) x ones.
    """
    n = xyz.shape[0]
    x32 = xyz.astype(np.float32)
    sq = (x32 * x32).sum(1)
    ones = np.ones(n, ml_dtypes.bfloat16)
    s_rows, t_rows = [], []
    for d in range(3):
        ah, am, al = _bf3(x32[:, d])
        bh, bm, bl = _bf3(-2.0 * x32[:, d])
        s_rows += [ah, ah, am, ah, al, am]
        t_rows += [bh, bm, bh, bl, bh, bm]
    nh, nm, nl = _bf3(sq)
    s_rows += [ones, ones, ones, nh, nm, nl]
    t_rows += [nh, nm, nl, ones, ones, ones]

    # replicate at base partitions 0/32/64/96 for PE row-group packing
    half = n // 2
    groups = min(4, half // min(512, half))
    s = np.zeros((32 * groups, n), ml_dtypes.bfloat16)
    t = np.zeros((32 * groups, n), ml_dtypes.bfloat16)
    for g in range(groups):
        s[32 * g : 32 * g + K_ROWS] = np.stack(s_rows)
        t[32 * g : 32 * g + K_ROWS] = np.stack(t_rows)
    return s, t


def _decode_step(counts, n_counted):
    """accum -> c = ceil(t*/2); handles sign(0) being 0 or +1 on HW."""
    if counts.max() > 0.5:  # sign(+0) == +1 convention
        c = (n_counted - counts) * 0.5
    else:  # sign(0) == 0
        c = -counts
    out = np.rint(c).astype(np.int64)
    np.clip(out, 0, n_counted, out=out)
    return out


def _resolve_idx(cstar, xa, xb, half):
    """Pick the true argmin among candidates {4c-3..4c} x {+0, +half} by
    exact distance (ties -> smallest index, matching jnp.argmin)."""
    ts = [np.clip(4 * cstar + o, 0, half - 1) for o in (-3, -2, -1, 0)]
    cand = np.stack(ts + [t + half for t in ts], 1)  # m-ascending
    dists = ((xa[:, None, :] - xb[cand]) ** 2).sum(-1)
    pick = dists.argmin(1)
    rows = np.arange(len(cstar))
    return cand[rows, pick], dists[rows, pick]


def kernel(xyz1, xyz2, normal_rebuild, normal_gt):
    nc = _compiled()

    in_maps = []
    for b in range(B):
        s1, t1 = _operands(np.asarray(xyz1[b], np.float32))
        s2, t2 = _operands(np.asarray(xyz2[b], np.float32))
        in_maps.append({"s1": s1, "t1": t1, "s2": s2, "t2": t2})

    res = run_bass_kernel_spmd(nc, in_maps, core_ids=list(range(B)))

    half = N_PTS // 2
    loss_xyz = 0.0
    loss_normal = 0.0
    for b in range(B):
        r = {k: np.ascontiguousarray(v.T).reshape(-1) for k, v in res.results[b].items()}
        t1s = _decode_step(r["c1"], half // 4)
        t2s = _decode_step(r["c2"], half // 4)
        x1 = np.asarray(xyz1[b]).astype(np.float64)
        x2 = np.asarray(xyz2[b]).astype(np.float64)
        idx1, dist1 = _resolve_idx(t1s, x1, x2, half)
        idx2, dist2 = _resolve_idx(t2s, x2, x1, half)
        loss_xyz += dist1.mean() + dist2.mean()

        def _norm(v):
            v = v.astype(np.float64)
            n = np.sqrt((v * v).sum(-1, keepdims=True))
            return v / np.maximum(n, 1e-12)

        a = _norm(np.asarray(normal_rebuild[b]))
        g = _norm(np.asarray(normal_gt[b]))
        t1n = g[idx1]
        t2n = a[idx2]
        nd1 = np.minimum(((a - t1n) ** 2).sum(-1), ((a + t1n) ** 2).sum(-1))
        nd2 = np.minimum(((g - t2n) ** 2).sum(-1), ((g + t2n) ** 2).sum(-1))
        loss_normal += nd1.mean() + nd2.mean()

    return (np.float32(loss_xyz / B), np.float32(loss_normal / B))
